# revision 21
# baseline (speedup 1.0000x reference)
"""DynamicMaskHead Trainium2 kernel.

Per-instance 3-layer MLP over pixels (grouped 1x1 convs):
    out = w2 @ relu(w1 @ relu(w0 @ x + b0) + b1) + b2
with 128 instances, x: [10, 25600] per instance.

Sharding: 16 instances per NeuronCore (8 cores, data-parallel, no
cross-core communication).

Per-core mapping (instances j in [0,16)):
  - L1 runs as two accumulating block-diagonal matmuls into one PSUM bank:
      K=128 over (j, cin 0..7)  partitions  p = 8j + k
      K=32  over (j, cin 8..9)  partitions 64 + 2j + (k-8)  (row strip 64..95)
    output partition m = 8j + o (16 instances x 8 channels).
  - L2: one K=128 block-diagonal matmul [128 -> 128].
  - L3: one K=128 matmul [128 -> 16] (one output channel per instance).
  - Matmul operands are bf16 (fp32 PSUM accumulate): full-rate PE with fast
    weight loads; activations/weights are rounded to bf16 on the host so the
    HBM stream is half-width too.
  - Bias+relu epilogues ride PSUM->SBUF on ScalarE (activation w/ bias AP)
    and VectorE (tensor_scalar add+max), batched over two PSUM banks where
    possible and balanced across the two engines.
  - Pixel tiles are processed in groups of 4 per weight set so the PE gets
    long uninterrupted matmul bursts (HAM warm-up) and weight loads amortize.
  - DMA queues split by purpose: weights on ACT(HWDGE), x on SP(HWDGE),
    out on GpSimd(SWDGE) so input streaming is never stuck behind waits.
"""

import sys

if "/opt/trn_rl_repo" not in sys.path:
    sys.path.insert(0, "/opt/trn_rl_repo")

import ml_dtypes
import numpy as np

N_CORES = 8
N_INST = 128
C_IN = 10
C = 8
H = W = 160
P = H * W          # 25600 pixels
PER = N_INST // N_CORES  # 16 instances per core
F = 512            # pixels per matmul tile (one fp32 PSUM bank)
NTILE = P // F     # 50
SUP = 5120         # pixels per DMA super-tile (10KB/partition in bf16)
NSUP = P // SUP    # 5
TPS = SUP // F     # 10 tiles per super-tile
GROUP = 4          # tiles per weight-load group

_cached_nc = None


def _build():
    from concourse import bacc, bass, mybir, tile

    nc = bacc.Bacc("TRN2", target_bir_lowering=False, debug=False)
    f32 = mybir.dt.float32
    bf16 = mybir.dt.bfloat16
    Relu = mybir.ActivationFunctionType.Relu
    Ident = mybir.ActivationFunctionType.Identity
    op_add = mybir.AluOpType.add
    op_max = mybir.AluOpType.max

    x1_d = nc.dram_tensor("x1", [128, P], bf16, kind="ExternalInput")
    x2_d = nc.dram_tensor("x2", [32, P], bf16, kind="ExternalInput")
    w1a_d = nc.dram_tensor("w1a", [128, 128], bf16, kind="ExternalInput")
    w1b_d = nc.dram_tensor("w1b", [32, 128], bf16, kind="ExternalInput")
    w2_d = nc.dram_tensor("w2", [128, 128], bf16, kind="ExternalInput")
    w3_d = nc.dram_tensor("w3", [128, 16], bf16, kind="ExternalInput")
    b0_d = nc.dram_tensor("b0", [128, 1], f32, kind="ExternalInput")
    b1_d = nc.dram_tensor("b1", [128, 1], f32, kind="ExternalInput")
    b2r_d = nc.dram_tensor("b2r", [128, 1], f32, kind="ExternalInput")
    out_d = nc.dram_tensor("out", [PER, P], f32, kind="ExternalOutput")

    with tile.TileContext(nc) as tc:
        with (
            tc.tile_pool(name="const", bufs=1) as cpool,
            tc.tile_pool(name="xp", bufs=3) as xpool,
            tc.tile_pool(name="hp", bufs=6) as hpool,
            tc.tile_pool(name="op", bufs=2) as opool,
            tc.tile_pool(name="ps1", bufs=4, space="PSUM") as pp1,
            tc.tile_pool(name="ps2", bufs=3, space="PSUM") as pp2,
            tc.tile_pool(name="ps3", bufs=1, space="PSUM") as pp3,
        ):
            w1a = cpool.tile([128, 128], bf16)
            nc.scalar.dma_start(w1a[:], w1a_d[:])
            w1b = cpool.tile([128, 128], bf16)
            for k in range(4):
                nc.scalar.dma_start(w1b[32 * k : 32 * k + 32, :], w1b_d[:])
            w2t = cpool.tile([128, 128], bf16)
            nc.scalar.dma_start(w2t[:], w2_d[:])
            w3t = cpool.tile([128, 16], bf16)
            nc.scalar.dma_start(w3t[:], w3_d[:])
            b0t = cpool.tile([128, 1], f32)
            nc.scalar.dma_start(b0t[:], b0_d[:])
            b1t = cpool.tile([128, 1], f32)
            nc.scalar.dma_start(b1t[:], b1_d[:])
            b2rt = cpool.tile([128, 1], f32)
            nc.scalar.dma_start(b2rt[:], b2r_d[:])

            # PE warm-up: ~10 dummy matmuls on garbage data while the first
            # x DMAs are in flight, so HAM un-throttles before real work.
            wdum = cpool.tile([128, 128], bf16, name="wdum")
            nc.vector.memset(wdum[:], 0.0)
            xdum = cpool.tile([128, F], bf16, name="xdum")
            nc.vector.memset(xdum[:], 0.0)
            for wi in range(10):
                psw = pp1.tile([128, F], f32, name="psw", tag="ps1")
                nc.tensor.matmul(
                    psw[:], wdum[:], xdum[:], start=True, stop=True
                )

            # x super-tile SBUF tiles, filled on demand, indexed by super id.
            # x2 lives in four 32-partition strips (strip = within-super tile
            # index mod 4) so the four K=32 L1b matmuls of a group run
            # concurrently on disjoint PE row strips.
            NSLOT = (TPS + 3) // 4
            xtiles = {}

            def get_x(s):
                if s not in xtiles:
                    x1 = xpool.tile([128, SUP], bf16, name="x1", tag="x1")
                    half = SUP // 2
                    nc.sync.dma_start(
                        x1[:, 0:half], x1_d[:, s * SUP : s * SUP + half]
                    )
                    nc.sync.dma_start(
                        x1[:, half:SUP], x1_d[:, s * SUP + half : (s + 1) * SUP]
                    )
                    x2 = xpool.tile([128, NSLOT * F], bf16, name="x2", tag="x2")
                    src3 = x2_d[:, s * SUP : (s + 1) * SUP].rearrange(
                        "p (r c) -> p r c", r=TPS
                    )
                    for k in range(4):
                        n_k = len(range(k, TPS, 4))
                        nc.sync.dma_start(
                            x2[32 * k : 32 * k + 32, :].rearrange(
                                "p (q c) -> p q c", q=NSLOT
                            )[:, 0:n_k, :],
                            src3[:, k::4, :],
                        )
                    xtiles[s] = (x1, x2)
                return xtiles[s]

            # out staging: one chunk tile holds 4 groups (16 pixel tiles) in
            # strip layout; strip DMAs unscramble to [16, P] DRAM layout.
            ochunks = {}

            def get_ochunk(c):
                if c not in ochunks:
                    ochunks[c] = opool.tile([128, 4 * F], f32, name="oc", tag="out")
                return ochunks[c]

            def xcol(t):
                s, r = divmod(t, TPS)
                x1, x2 = get_x(s)
                return x1, x2, r

            # 3-stage software pipeline over 2-tile groups: the PE runs
            # group g's L1 while group g-1's L2 and group g-2's L3 execute,
            # so the cross-engine epilogue latency (A1 on ACT, A2 on DVE)
            # never stalls the dependent matmuls.
            NG = (NTILE + GROUP - 1) // GROUP  # 13 groups of 4 tiles
            st = {}

            def stage_l1(g):
                tiles = list(range(GROUP * g, min(GROUP * (g + 1), NTILE)))
                ps1s = {}
                for t in tiles:
                    ps1s[t] = pp1.tile([128, F], f32, name="ps1", tag="ps1")
                for t in tiles:
                    x1, _, r = xcol(t)
                    nc.tensor.matmul(
                        ps1s[t][:], w1a[:], x1[:, bass.ts(r, F)],
                        start=True, stop=False,
                    )
                for t in tiles:
                    _, x2, r = xcol(t)
                    strip, slot = r % 4, r // 4
                    nc.tensor.matmul(
                        ps1s[t][:],
                        w1b[32 * strip : 32 * strip + 32, :],
                        x2[32 * strip : 32 * strip + 32, bass.ts(slot, F)],
                        start=False,
                        stop=True,
                        tile_position=(32 * strip, 0),
                    )
                h1s = {}
                for t in tiles:
                    h1s[t] = hpool.tile([128, F], bf16, name="h1", tag="h1")
                    if t % 8 == 7:
                        nc.vector.tensor_scalar(
                            h1s[t][:], ps1s[t][:], b0t[:], 0.0,
                            op0=op_add, op1=op_max,
                        )
                    else:
                        nc.scalar.activation(
                            h1s[t][:], ps1s[t][:], Relu, bias=b0t[:]
                        )
                st[g] = {"h1s": h1s}

            def stage_l2(g):
                h1s = st[g]["h1s"]
                h2s = {}
                for t in sorted(h1s):
                    ps2 = pp2.tile([128, F], f32, name="ps2", tag="ps2")
                    nc.tensor.matmul(
                        ps2[:], w2t[:], h1s[t][:], start=True, stop=True
                    )
                    h2s[t] = (hpool.tile([128, F], bf16, name="h2", tag="h2"), ps2)
                for t in sorted(h2s):
                    h2, ps2 = h2s[t]
                    nc.vector.tensor_scalar(
                        h2[:], ps2[:], b1t[:], 0.0, op0=op_add, op1=op_max
                    )
                st[g]["h2s"] = {t: v[0] for t, v in h2s.items()}

            def stage_l3(g):
                h2s = st[g]["h2s"]
                ps3 = pp3.tile([128, F], f32, name="ps3", tag="ps3")
                for t in sorted(h2s):
                    k = t % 4
                    nc.tensor.matmul(
                        ps3[32 * k : 32 * k + 16, :], w3t[:], h2s[t][:],
                        start=True, stop=True,
                        tile_position=(0, 32 * k),
                    )
                last_tile = max(h2s)
                c, qc = divmod(g, 4)
                oc = get_ochunk(c)
                nrows = 32 * (last_tile % 4) + 16
                nc.scalar.activation(
                    oc[0:nrows, bass.ts(qc, F)], ps3[0:nrows, :],
                    Ident, bias=b2rt[0:nrows, :],
                )
                chunk_done = (qc == 3) or (last_tile == NTILE - 1)
                if not chunk_done:
                    return
                nq = qc + 1
                base = c * 16 * F
                if nq == 4:
                    dram4 = out_d[:, base : base + 16 * F].rearrange(
                        "j (m four c) -> j m four c", m=4, four=4
                    )
                    for k in range(4):
                        nc.gpsimd.dma_start(
                            dram4[:, :, k, :],
                            oc[32 * k : 32 * k + 16, :].rearrange(
                                "j (m c) -> j m c", m=4
                            ),
                        )
                else:
                    for qq in range(nq):
                        for k4 in range(4):
                            t = (c * 4 + qq) * 4 + k4
                            if t >= NTILE:
                                break
                            nc.gpsimd.dma_start(
                                out_d[:, t * F : (t + 1) * F],
                                oc[32 * k4 : 32 * k4 + 16, bass.ts(qq, F)],
                            )

            for i in range(NG + 2):
                if i < NG:
                    stage_l1(i)
                if 0 <= i - 1 < NG:
                    stage_l2(i - 1)
                if 0 <= i - 2 < NG:
                    stage_l3(i - 2)
                    del st[i - 2]

    nc.compile()
    return nc


def _prep_inputs(features, params):
    feats = np.ascontiguousarray(features, dtype=np.float32).reshape(N_INST, C_IN, P)
    params = np.asarray(params, dtype=np.float32)
    bf = ml_dtypes.bfloat16
    in_maps = []
    for c in range(N_CORES):
        js = slice(c * PER, (c + 1) * PER)
        pc = params[js]
        w0 = pc[:, :80].reshape(PER, C, C_IN)
        w1 = pc[:, 80:144].reshape(PER, C, C)
        w2 = pc[:, 144:152].reshape(PER, 1, C)
        b0 = pc[:, 152:160]
        b1 = pc[:, 160:168]
        b2 = pc[:, 168:169]
        w1a = np.zeros((128, 128), np.float32)
        w1b = np.zeros((32, 128), np.float32)
        w2b = np.zeros((128, 128), np.float32)
        w3b = np.zeros((128, 16), np.float32)
        for j in range(PER):
            w1a[j * 8 : j * 8 + 8, j * 8 : j * 8 + 8] = w0[j, :, :8].T
            w1b[j * 2 : j * 2 + 2, j * 8 : j * 8 + 8] = w0[j, :, 8:10].T
            w2b[j * 8 : j * 8 + 8, j * 8 : j * 8 + 8] = w1[j].T
            w3b[j * 8 : j * 8 + 8, j] = w2[j, 0, :]
        b2rep = np.zeros((128, 1), np.float32)
        for k in range(4):
            b2rep[32 * k : 32 * k + 16, 0] = b2[:, 0]
        x = feats[js]
        in_maps.append(
            {
                "x1": np.ascontiguousarray(x[:, :8, :]).reshape(128, P).astype(bf),
                "x2": np.ascontiguousarray(x[:, 8:, :]).reshape(32, P).astype(bf),
                "w1a": w1a.astype(bf),
                "w1b": w1b.astype(bf),
                "w2": w2b.astype(bf),
                "w3": w3b.astype(bf),
                "b0": np.ascontiguousarray(b0).reshape(128, 1),
                "b1": np.ascontiguousarray(b1).reshape(128, 1),
                "b2r": b2rep,
            }
        )
    return in_maps


def _run(features, params, trace=False, **kwargs):
    global _cached_nc
    from concourse.bass_utils import run_bass_kernel_spmd

    if _cached_nc is None:
        _cached_nc = _build()
    in_maps = _prep_inputs(features, params)
    res = run_bass_kernel_spmd(
        _cached_nc, in_maps, list(range(N_CORES)), trace=trace, **kwargs
    )
    out = np.empty((N_INST, 1, H, W), np.float32)
    for c in range(N_CORES):
        out[c * PER : (c + 1) * PER, 0] = res.results[c]["out"].reshape(PER, H, W)
    return out, res


def kernel(features, params, num_insts=None, **_ignored):
    out, _ = _run(features, params, trace=False)
    return out


# revision 22
# speedup vs baseline: 1.0504x; 1.0504x over previous
"""DynamicMaskHead Trainium2 kernel.

Per-instance 3-layer MLP over pixels (grouped 1x1 convs):
    out = w2 @ relu(w1 @ relu(w0 @ x + b0) + b1) + b2
with 128 instances, x: [10, 25600] per instance.

Sharding: 16 instances per NeuronCore (8 cores, data-parallel, no
cross-core communication).

Per-core mapping (instances j in [0,16)):
  - L1 runs as two accumulating block-diagonal matmuls into one PSUM bank:
      K=128 over (j, cin 0..7)  partitions  p = 8j + k
      K=32  over (j, cin 8..9)  partitions 64 + 2j + (k-8)  (row strip 64..95)
    output partition m = 8j + o (16 instances x 8 channels).
  - L2: one K=128 block-diagonal matmul [128 -> 128].
  - L3: one K=128 matmul [128 -> 16] (one output channel per instance).
  - Matmul operands are bf16 (fp32 PSUM accumulate): full-rate PE with fast
    weight loads; activations/weights are rounded to bf16 on the host so the
    HBM stream is half-width too.
  - Bias+relu epilogues ride PSUM->SBUF on ScalarE (activation w/ bias AP)
    and VectorE (tensor_scalar add+max), batched over two PSUM banks where
    possible and balanced across the two engines.
  - Pixel tiles are processed in groups of 4 per weight set so the PE gets
    long uninterrupted matmul bursts (HAM warm-up) and weight loads amortize.
  - DMA queues split by purpose: weights on ACT(HWDGE), x on SP(HWDGE),
    out on GpSimd(SWDGE) so input streaming is never stuck behind waits.
"""

import sys

if "/opt/trn_rl_repo" not in sys.path:
    sys.path.insert(0, "/opt/trn_rl_repo")

import ml_dtypes
import numpy as np

N_CORES = 8
N_INST = 128
C_IN = 10
C = 8
H = W = 160
P = H * W          # 25600 pixels
PER = N_INST // N_CORES  # 16 instances per core
F = 512            # pixels per matmul tile (one fp32 PSUM bank)
NTILE = P // F     # 50
SUP = 5120         # pixels per DMA super-tile (10KB/partition in bf16)
NSUP = P // SUP    # 5
TPS = SUP // F     # 10 tiles per super-tile
GROUP = 4          # tiles per weight-load group

_cached_nc = None


def _build():
    from concourse import bacc, bass, mybir, tile

    nc = bacc.Bacc("TRN2", target_bir_lowering=False, debug=False)
    f32 = mybir.dt.float32
    bf16 = mybir.dt.bfloat16
    Relu = mybir.ActivationFunctionType.Relu
    Ident = mybir.ActivationFunctionType.Identity
    op_add = mybir.AluOpType.add
    op_max = mybir.AluOpType.max

    x1_d = nc.dram_tensor("x1", [128, P], bf16, kind="ExternalInput")
    x2_d = nc.dram_tensor("x2", [32, P], bf16, kind="ExternalInput")
    w1a_d = nc.dram_tensor("w1a", [128, 128], bf16, kind="ExternalInput")
    w1b_d = nc.dram_tensor("w1b", [32, 128], bf16, kind="ExternalInput")
    w2_d = nc.dram_tensor("w2", [128, 128], bf16, kind="ExternalInput")
    w3_d = nc.dram_tensor("w3", [128, 16], bf16, kind="ExternalInput")
    b0_d = nc.dram_tensor("b0", [128, 1], f32, kind="ExternalInput")
    b1_d = nc.dram_tensor("b1", [128, 1], f32, kind="ExternalInput")
    b2r_d = nc.dram_tensor("b2r", [128, 1], f32, kind="ExternalInput")
    out_d = nc.dram_tensor("out", [PER, P], f32, kind="ExternalOutput")

    with tile.TileContext(nc) as tc:
        with (
            tc.tile_pool(name="const", bufs=1) as cpool,
            tc.tile_pool(name="xp", bufs=4) as xpool,
            tc.tile_pool(name="hp", bufs=10) as hpool,
            tc.tile_pool(name="op", bufs=2) as opool,
            tc.tile_pool(name="ps1", bufs=4, space="PSUM") as pp1,
            tc.tile_pool(name="ps2", bufs=3, space="PSUM") as pp2,
            tc.tile_pool(name="ps3", bufs=1, space="PSUM") as pp3,
        ):
            w1a = cpool.tile([128, 128], bf16)
            nc.scalar.dma_start(w1a[:], w1a_d[:])
            w1b = cpool.tile([128, 128], bf16)
            for k in range(4):
                nc.scalar.dma_start(w1b[32 * k : 32 * k + 32, :], w1b_d[:])
            w2t = cpool.tile([128, 128], bf16)
            nc.scalar.dma_start(w2t[:], w2_d[:])
            w3t = cpool.tile([128, 16], bf16)
            nc.scalar.dma_start(w3t[:], w3_d[:])
            b0t = cpool.tile([128, 1], f32)
            nc.scalar.dma_start(b0t[:], b0_d[:])
            b1t = cpool.tile([128, 1], f32)
            nc.scalar.dma_start(b1t[:], b1_d[:])
            b2rt = cpool.tile([128, 1], f32)
            nc.scalar.dma_start(b2rt[:], b2r_d[:])

            # PE warm-up: ~10 dummy matmuls on garbage data while the first
            # x DMAs are in flight, so HAM un-throttles before real work.
            wdum = cpool.tile([128, 128], bf16, name="wdum")
            nc.vector.memset(wdum[:], 0.0)
            xdum = cpool.tile([128, F], bf16, name="xdum")
            nc.vector.memset(xdum[:], 0.0)
            for wi in range(10):
                psw = pp1.tile([128, F], f32, name="psw", tag="ps1")
                nc.tensor.matmul(
                    psw[:], wdum[:], xdum[:], start=True, stop=True
                )

            # x super-tile SBUF tiles, filled on demand, indexed by super id.
            # x2 lives in four 32-partition strips (strip = within-super tile
            # index mod 4) so the four K=32 L1b matmuls of a group run
            # concurrently on disjoint PE row strips.
            NSLOT = (TPS + 3) // 4
            xtiles = {}

            def get_x(s):
                if s not in xtiles:
                    x1 = xpool.tile([128, SUP], bf16, name="x1", tag="x1")
                    half = SUP // 2
                    nc.sync.dma_start(
                        x1[:, 0:half], x1_d[:, s * SUP : s * SUP + half]
                    )
                    nc.sync.dma_start(
                        x1[:, half:SUP], x1_d[:, s * SUP + half : (s + 1) * SUP]
                    )
                    x2 = xpool.tile([128, NSLOT * F], bf16, name="x2", tag="x2")
                    src3 = x2_d[:, s * SUP : (s + 1) * SUP].rearrange(
                        "p (r c) -> p r c", r=TPS
                    )
                    for k in range(4):
                        n_k = len(range(k, TPS, 4))
                        nc.sync.dma_start(
                            x2[32 * k : 32 * k + 32, :].rearrange(
                                "p (q c) -> p q c", q=NSLOT
                            )[:, 0:n_k, :],
                            src3[:, k::4, :],
                        )
                    xtiles[s] = (x1, x2)
                return xtiles[s]

            # out staging: one chunk tile holds 4 groups (16 pixel tiles) in
            # strip layout; strip DMAs unscramble to [16, P] DRAM layout.
            ochunks = {}

            def get_ochunk(c):
                if c not in ochunks:
                    ochunks[c] = opool.tile([128, 4 * F], f32, name="oc", tag="out")
                return ochunks[c]

            def xcol(t):
                s, r = divmod(t, TPS)
                x1, x2 = get_x(s)
                return x1, x2, r

            # 3-stage software pipeline over 2-tile groups: the PE runs
            # group g's L1 while group g-1's L2 and group g-2's L3 execute,
            # so the cross-engine epilogue latency (A1 on ACT, A2 on DVE)
            # never stalls the dependent matmuls.
            NG = NTILE // 2  # 25 groups of 2 tiles
            st = {}

            def stage_l1(g):
                tiles = [2 * g, 2 * g + 1]
                ps1s = {}
                for t in tiles:
                    ps1s[t] = pp1.tile([128, F], f32, name="ps1", tag="ps1")
                for t in tiles:
                    x1, _, r = xcol(t)
                    nc.tensor.matmul(
                        ps1s[t][:], w1a[:], x1[:, bass.ts(r, F)],
                        start=True, stop=False,
                    )
                for t in tiles:
                    _, x2, r = xcol(t)
                    strip, slot = r % 4, r // 4
                    nc.tensor.matmul(
                        ps1s[t][:],
                        w1b[32 * strip : 32 * strip + 32, :],
                        x2[32 * strip : 32 * strip + 32, bass.ts(slot, F)],
                        start=False,
                        stop=True,
                        tile_position=(32 * strip, 0),
                    )
                h1s = {}
                for t in tiles:
                    h1s[t] = hpool.tile([128, F], bf16, name="h1", tag="h1")
                    if t % 8 == 7:
                        nc.vector.tensor_scalar(
                            h1s[t][:], ps1s[t][:], b0t[:], 0.0,
                            op0=op_add, op1=op_max,
                        )
                    else:
                        nc.scalar.activation(
                            h1s[t][:], ps1s[t][:], Relu, bias=b0t[:]
                        )
                st[g] = {"h1s": h1s}

            def stage_l2(g):
                h1s = st[g]["h1s"]
                h2s = {}
                for t in sorted(h1s):
                    ps2 = pp2.tile([128, F], f32, name="ps2", tag="ps2")
                    nc.tensor.matmul(
                        ps2[:], w2t[:], h1s[t][:], start=True, stop=True
                    )
                    h2s[t] = (hpool.tile([128, F], bf16, name="h2", tag="h2"), ps2)
                for t in sorted(h2s):
                    h2, ps2 = h2s[t]
                    nc.vector.tensor_scalar(
                        h2[:], ps2[:], b1t[:], 0.0, op0=op_add, op1=op_max
                    )
                st[g]["h2s"] = {t: v[0] for t, v in h2s.items()}

            ps3cur = [None]

            def stage_l3(g):
                h2s = st[g]["h2s"]
                if g % 2 == 0:
                    ps3cur[0] = pp3.tile([128, F], f32, name="ps3", tag="ps3")
                ps3 = ps3cur[0]
                for t in sorted(h2s):
                    k = t % 4
                    nc.tensor.matmul(
                        ps3[32 * k : 32 * k + 16, :], w3t[:], h2s[t][:],
                        start=True, stop=True,
                        tile_position=(0, 32 * k),
                    )
                last_tile = max(h2s)
                quartet_done = (g % 2 == 1) or (last_tile == NTILE - 1)
                if not quartet_done:
                    return
                q = g // 2
                c, qc = divmod(q, 4)
                oc = get_ochunk(c)
                nrows = 32 * (last_tile % 4) + 16
                nc.scalar.activation(
                    oc[0:nrows, bass.ts(qc, F)], ps3[0:nrows, :],
                    Ident, bias=b2rt[0:nrows, :],
                )
                chunk_done = (qc == 3) or (last_tile == NTILE - 1)
                if not chunk_done:
                    return
                nq = qc + 1
                base = c * 16 * F
                if nq == 4:
                    dram4 = out_d[:, base : base + 16 * F].rearrange(
                        "j (m four c) -> j m four c", m=4, four=4
                    )
                    for k in range(4):
                        nc.gpsimd.dma_start(
                            dram4[:, :, k, :],
                            oc[32 * k : 32 * k + 16, :].rearrange(
                                "j (m c) -> j m c", m=4
                            ),
                        )
                else:
                    for qq in range(nq):
                        for k4 in range(4):
                            t = (c * 4 + qq) * 4 + k4
                            if t >= NTILE:
                                break
                            nc.gpsimd.dma_start(
                                out_d[:, t * F : (t + 1) * F],
                                oc[32 * k4 : 32 * k4 + 16, bass.ts(qq, F)],
                            )

            for i in range(NG + 2):
                if i < NG:
                    stage_l1(i)
                if 0 <= i - 1 < NG:
                    stage_l2(i - 1)
                if 0 <= i - 2 < NG:
                    stage_l3(i - 2)
                    del st[i - 2]

    nc.compile()
    return nc


def _prep_inputs(features, params):
    feats = np.ascontiguousarray(features, dtype=np.float32).reshape(N_INST, C_IN, P)
    params = np.asarray(params, dtype=np.float32)
    bf = ml_dtypes.bfloat16
    in_maps = []
    for c in range(N_CORES):
        js = slice(c * PER, (c + 1) * PER)
        pc = params[js]
        w0 = pc[:, :80].reshape(PER, C, C_IN)
        w1 = pc[:, 80:144].reshape(PER, C, C)
        w2 = pc[:, 144:152].reshape(PER, 1, C)
        b0 = pc[:, 152:160]
        b1 = pc[:, 160:168]
        b2 = pc[:, 168:169]
        w1a = np.zeros((128, 128), np.float32)
        w1b = np.zeros((32, 128), np.float32)
        w2b = np.zeros((128, 128), np.float32)
        w3b = np.zeros((128, 16), np.float32)
        for j in range(PER):
            w1a[j * 8 : j * 8 + 8, j * 8 : j * 8 + 8] = w0[j, :, :8].T
            w1b[j * 2 : j * 2 + 2, j * 8 : j * 8 + 8] = w0[j, :, 8:10].T
            w2b[j * 8 : j * 8 + 8, j * 8 : j * 8 + 8] = w1[j].T
            w3b[j * 8 : j * 8 + 8, j] = w2[j, 0, :]
        b2rep = np.zeros((128, 1), np.float32)
        for k in range(4):
            b2rep[32 * k : 32 * k + 16, 0] = b2[:, 0]
        x = feats[js]
        in_maps.append(
            {
                "x1": np.ascontiguousarray(x[:, :8, :]).reshape(128, P).astype(bf),
                "x2": np.ascontiguousarray(x[:, 8:, :]).reshape(32, P).astype(bf),
                "w1a": w1a.astype(bf),
                "w1b": w1b.astype(bf),
                "w2": w2b.astype(bf),
                "w3": w3b.astype(bf),
                "b0": np.ascontiguousarray(b0).reshape(128, 1),
                "b1": np.ascontiguousarray(b1).reshape(128, 1),
                "b2r": b2rep,
            }
        )
    return in_maps


def _run(features, params, trace=False, **kwargs):
    global _cached_nc
    from concourse.bass_utils import run_bass_kernel_spmd

    if _cached_nc is None:
        _cached_nc = _build()
    in_maps = _prep_inputs(features, params)
    res = run_bass_kernel_spmd(
        _cached_nc, in_maps, list(range(N_CORES)), trace=trace, **kwargs
    )
    out = np.empty((N_INST, 1, H, W), np.float32)
    for c in range(N_CORES):
        out[c * PER : (c + 1) * PER, 0] = res.results[c]["out"].reshape(PER, H, W)
    return out, res


def kernel(features, params, num_insts=None, **_ignored):
    out, _ = _run(features, params, trace=False)
    return out
